# revision 10
# baseline (speedup 1.0000x reference)
"""Trainium2 Bass kernel for histogram-binning NLL loss.

reference:
    probs: [N=32, T=256, K=8000] f32, targets: [N, L=64] int
    agg[n,k]    = sum_t (probs[n,t,k] + 1e-10)       = colsum[n,k] + T*1e-10
    count[n,k]  = histogram(targets[n]) over K
    loss        = mean_n( -sum_k log(agg/T) * count/T )
                = sum_{n,k} (-count[n,k]/(N*T)) * log(colsum[n,k]/T + 1e-10)

Sharding: pure data-parallel over N across 8 cores (4 rows each).
Each core:
  - streams its probs shard [4, 256, 8000] as 8 x [128, 8000] tiles
  - per row n: 63 chunk matmuls (lhsT=probs chunk [128, <=128] stationary,
    rhs=ones[128,1]) accumulate the T=256 reduction into PSUM [<=128, 1]
    columns -> colsum laid out K-on-partitions
  - ScalarE: log(colsum/T + 1e-10) from PSUM
  - VectorE: multiply by host-precomputed weights -count/(N*T), reduce
  -> out [128, 1]; host sums the 8 x 128 partials.
"""

import numpy as np

N, T, K = 32, 256, 8000
L = 64
NCORES = 8
NS = N // NCORES  # rows per core = 4
P = 128
CH = (K + P - 1) // P  # 63 column chunks per row
SOFT = 1e-10

_cached = {}


def _build_nc():
    from contextlib import ExitStack

    import concourse.bass as bass
    import concourse.mybir as mybir

    nc = bass.Bass()
    probs = nc.declare_dram_parameter(
        "probs", [NS, T, K], mybir.dt.float32, isOutput=False
    )
    wts = nc.declare_dram_parameter(
        "wts", [P, NS * CH], mybir.dt.float32, isOutput=False
    )
    out = nc.declare_dram_parameter("out", [P, 1], mybir.dt.float32, isOutput=True)

    # Register const APs (behind an all-engine barrier) so matmul rhs /
    # activation bias+scale carry no semaphore deps.
    for v in (SOFT, 1.0 / T):
        t = nc.alloc_sbuf_tensor(f"const-f32-{v}", [128, 1], mybir.dt.float32)
        nc.gpsimd.memset(t.ap(), v)
        nc.const_aps.aps[(mybir.dt.float32, v)] = t.ap()
    nc.all_engine_barrier()
    ones = nc.const_aps.tensor(1.0, (P, 1), mybir.dt.float32)

    # [NS*T, K] -> [NS*2, 128, K] tiles; tile 2n/2n+1 = row n's two T halves
    ptiles = probs[:].rearrange("n (j p) k -> (n j) p k", p=P)

    NT = 2 * NS  # 8 load tiles
    NBUF = 4  # resident load buffers
    full = CH - 1
    tail = K - full * P

    ctx = ExitStack()
    with ctx:
        bufs = [
            ctx.enter_context(nc.sbuf_tensor(f"buf{i}", [P, K], mybir.dt.float32))
            for i in range(NBUF)
        ]
        wtile = ctx.enter_context(
            nc.sbuf_tensor("wtile", [P, NS * CH], mybir.dt.float32)
        )
        logt = ctx.enter_context(
            nc.sbuf_tensor("logt", [P, NS * CH], mybir.dt.float32)
        )
        prod = ctx.enter_context(
            nc.sbuf_tensor("prod", [P, NS * CH], mybir.dt.float32)
        )
        acc = ctx.enter_context(nc.sbuf_tensor("acc", [P, 1], mybir.dt.float32))
        pss = [
            ctx.enter_context(nc.psum_tensor(f"ps{i}", [P, CH], mybir.dt.float32))
            for i in range(NS)
        ]
        s_buf = [ctx.enter_context(nc.semaphore(f"s_buf{i}")) for i in range(NBUF)]
        s_w = ctx.enter_context(nc.semaphore("s_w"))
        s_out = ctx.enter_context(nc.semaphore("s_out"))
        pe_sem = ctx.enter_context(nc.semaphore("pe_sem"))
        act_sem = ctx.enter_context(nc.semaphore("act_sem"))
        dve_sem = ctx.enter_context(nc.semaphore("dve_sem"))

        # ---- SYNC engine: all input DMAs (HWDGE FIFO), 0 or 1 wait each ----
        nc.sync.dma_start(out=wtile[:], in_=wts[:]).then_inc(s_w, 16)
        for i in range(NT):
            b = i % NBUF
            if i >= NBUF:
                # slot reuse: wait until PE finished the row-pair that used it
                nc.sync.wait_ge(pe_sem, (i - NBUF) // 2 + 1)
            nc.sync.dma_start(out=bufs[b][:], in_=ptiles[i]).then_inc(
                s_buf[b], 16
            )
        nc.sync.wait_ge(dve_sem, 1)
        nc.sync.dma_start(out=out[:], in_=acc[:]).then_inc(s_out, 16)
        nc.sync.wait_ge(s_out, 16)

        # ---- PE: per row, 63 col-sum matmul pairs (T halves accumulate) ----
        for n in range(NS):
            ia, ib = 2 * n, 2 * n + 1
            ba, bb = bufs[ia % NBUF], bufs[ib % NBUF]
            nc.tensor.wait_ge(s_buf[ia % NBUF], 16 * (ia // NBUF + 1))
            nc.tensor.wait_ge(s_buf[ib % NBUF], 16 * (ib // NBUF + 1))
            for c in range(CH):
                w = min(P, K - c * P)
                nc.tensor.matmul(
                    out=pss[n][:w, c : c + 1],
                    lhsT=ba[:, c * P : c * P + w],
                    rhs=ones[:, :1],
                    start=True,
                    stop=False,
                )
                mm = nc.tensor.matmul(
                    out=pss[n][:w, c : c + 1],
                    lhsT=bb[:, c * P : c * P + w],
                    rhs=ones[:, :1],
                    start=False,
                    stop=True,
                )
                if c == CH - 1:
                    mm.then_inc(pe_sem, 1)

        # ---- ACT: per row, log(colsum/T + eps) from PSUM ----
        for n in range(NS):
            nc.scalar.wait_ge(pe_sem, n + 1)
            nc.scalar.activation(
                out=logt[:, n * CH : n * CH + full],
                in_=pss[n][:, 0:full],
                func=mybir.ActivationFunctionType.Ln,
                bias=SOFT,
                scale=1.0 / T,
            )
            nc.scalar.activation(
                out=logt[:tail, n * CH + full : n * CH + full + 1],
                in_=pss[n][:tail, full : full + 1],
                func=mybir.ActivationFunctionType.Ln,
                bias=SOFT,
                scale=1.0 / T,
            ).then_inc(act_sem, 1)

        # ---- DVE: zero tail partitions, then weighted reduce ----
        for n in range(NS):
            nc.vector.memset(
                logt[tail:P, n * CH + full : n * CH + full + 1], 0.0
            )
        nc.vector.wait_ge(s_w, 16)
        nc.vector.wait_ge(act_sem, NS)
        nc.vector.tensor_tensor(
            out=prod[:], in0=logt[:], in1=wtile[:], op=mybir.AluOpType.mult
        )
        nc.vector.reduce_sum(
            out=acc[:], in_=prod[:], axis=mybir.AxisListType.X
        ).then_inc(dve_sem, 1)
    return nc


def _get_nc():
    if "nc" not in _cached:
        _cached["nc"] = _build_nc()
    return _cached["nc"]


def _make_wts(targets_shard: np.ndarray) -> np.ndarray:
    """[NS, L] int -> [P, NS*CH] f32 with w[p, n*CH+c] = -count[n, c*128+p]/(N*T)."""
    w = np.zeros((P, NS * CH), np.float32)
    for n in range(NS):
        cnt = np.bincount(
            targets_shard[n].astype(np.int64), minlength=CH * P
        ).astype(np.float32)
        w[:, n * CH : (n + 1) * CH] = (-cnt / (N * T)).reshape(CH, P).T
    return w


def kernel(**inputs) -> np.ndarray:
    from concourse.bass_utils import run_bass_kernel_spmd

    probs = np.ascontiguousarray(np.asarray(inputs["probs"], dtype=np.float32))
    targets = np.asarray(inputs["targets"])

    nc = _get_nc()
    in_maps = []
    for c in range(NCORES):
        sl = slice(c * NS, (c + 1) * NS)
        in_maps.append(
            {"probs": probs[sl], "wts": _make_wts(np.asarray(targets[sl]))}
        )
    res = run_bass_kernel_spmd(nc, in_maps, core_ids=list(range(NCORES))).results
    total = np.float64(0.0)
    for r in res:
        total += np.sum(np.asarray(r["out"], dtype=np.float64))
    return np.array(total, dtype=np.float32)


# revision 11
# speedup vs baseline: 1.9781x; 1.9781x over previous
"""Trainium2 Bass kernel for histogram-binning NLL loss.

reference:
    probs: [N=32, T=256, K=8000] f32, targets: [N, L=64] int
    agg[n,k]    = sum_t (probs[n,t,k] + 1e-10)       = colsum[n,k] + T*1e-10
    count[n,k]  = histogram(targets[n]) over K
    loss        = mean_n( -sum_k log(agg/T) * count/T )
                = sum_{n,k} (-count[n,k]/(N*T)) * log(colsum[n,k]/T + 1e-10)

Sharding: pure data-parallel over N across 8 cores (4 rows each).
Each core:
  - streams its probs shard [4, 256, 8000] as 8 x [128, 8000] tiles
  - per row n: 63 chunk matmuls (lhsT=probs chunk [128, <=128] stationary,
    rhs=ones[128,1]) accumulate the T=256 reduction into PSUM [<=128, 1]
    columns -> colsum laid out K-on-partitions
  - ScalarE: log(colsum/T + 1e-10) from PSUM
  - VectorE: multiply by host-precomputed weights -count/(N*T), reduce
  -> out [128, 1]; host sums the 8 x 128 partials.
"""

import numpy as np

N, T, K = 32, 256, 8000
L = 64
NCORES = 8
NS = N // NCORES  # rows per core = 4
P = 128
CH = (K + P - 1) // P  # 63 column chunks per row
SOFT = 1e-10

_cached = {}


def _build_nc():
    from contextlib import ExitStack

    import concourse.bass as bass
    import concourse.mybir as mybir

    nc = bass.Bass()
    probs = nc.declare_dram_parameter(
        "probs", [NS, T, K], mybir.dt.float32, isOutput=False
    )
    wts = nc.declare_dram_parameter(
        "wts", [P, NS * CH], mybir.dt.float32, isOutput=False
    )
    out = nc.declare_dram_parameter("out", [P, 1], mybir.dt.float32, isOutput=True)

    # Register const APs (behind an all-engine barrier) so matmul rhs /
    # activation bias+scale carry no semaphore deps.
    for v in (SOFT, 1.0 / T):
        t = nc.alloc_sbuf_tensor(f"const-f32-{v}", [128, 1], mybir.dt.float32)
        nc.gpsimd.memset(t.ap(), v)
        nc.const_aps.aps[(mybir.dt.float32, v)] = t.ap()
    nc.all_engine_barrier()
    ones = nc.const_aps.tensor(1.0, (P, 1), mybir.dt.float32)

    # [NS*T, K] -> [NS*2, 128, K] tiles; tile 2n/2n+1 = row n's two T halves
    ptiles = probs[:].rearrange("n (j p) k -> (n j) p k", p=P)

    NT = 2 * NS  # 8 load tiles
    NBUF = 4  # resident load buffers
    full = CH - 1
    tail = K - full * P

    ones_bf = nc.const_aps.tensor(1.0, (P, 1), mybir.dt.bfloat16)

    ctx = ExitStack()
    with ctx:
        bufs = [
            ctx.enter_context(nc.sbuf_tensor(f"buf{i}", [P, K], mybir.dt.float32))
            for i in range(NBUF)
        ]
        # bf16 per-row T-reduced tiles (DVE add of the two T halves)
        accb = [
            ctx.enter_context(nc.sbuf_tensor(f"accb{i}", [P, K], mybir.dt.bfloat16))
            for i in range(2)
        ]
        wtile = ctx.enter_context(
            nc.sbuf_tensor("wtile", [P, NS * CH], mybir.dt.float32)
        )
        logt = ctx.enter_context(
            nc.sbuf_tensor("logt", [P, NS * CH], mybir.dt.float32)
        )
        prod = ctx.enter_context(
            nc.sbuf_tensor("prod", [P, NS * CH], mybir.dt.float32)
        )
        acc = ctx.enter_context(nc.sbuf_tensor("acc", [P, 1], mybir.dt.float32))
        pss = [
            ctx.enter_context(nc.psum_tensor(f"ps{i}", [P, CH], mybir.dt.float32))
            for i in range(NS)
        ]
        s_buf = [ctx.enter_context(nc.semaphore(f"s_buf{i}")) for i in range(NBUF)]
        s_w = ctx.enter_context(nc.semaphore("s_w"))
        s_out = ctx.enter_context(nc.semaphore("s_out"))
        s_add = ctx.enter_context(nc.semaphore("s_add"))
        pe_sem = ctx.enter_context(nc.semaphore("pe_sem"))
        act_sem = ctx.enter_context(nc.semaphore("act_sem"))
        s_fin = ctx.enter_context(nc.semaphore("s_fin"))

        # ---- SYNC engine: all input DMAs (HWDGE FIFO), 0 or 1 wait each ----
        nc.sync.dma_start(out=wtile[:], in_=wts[:]).then_inc(s_w, 16)
        for i in range(NT):
            b = i % NBUF
            if i >= NBUF:
                # slot reuse: freed once DVE's add for that row-pair is done
                nc.sync.wait_ge(s_add, (i - NBUF) // 2 + 1)
            nc.sync.dma_start(out=bufs[b][:], in_=ptiles[i]).then_inc(
                s_buf[b], 16
            )
        nc.sync.wait_ge(s_fin, 1)
        nc.sync.dma_start(out=out[:], in_=acc[:]).then_inc(s_out, 16)
        nc.sync.wait_ge(s_out, 16)

        # ---- DVE: per row, add the two T halves, cast to bf16 ----
        for n in range(NS):
            ia, ib = 2 * n, 2 * n + 1
            nc.vector.wait_ge(s_buf[ia % NBUF], 16 * (ia // NBUF + 1))
            nc.vector.wait_ge(s_buf[ib % NBUF], 16 * (ib // NBUF + 1))
            if n >= 2:
                # accb slot reuse: PE must be done with row n-2
                nc.vector.wait_ge(pe_sem, n - 1)
            nc.vector.tensor_tensor(
                out=accb[n % 2][:],
                in0=bufs[ia % NBUF][:],
                in1=bufs[ib % NBUF][:],
                op=mybir.AluOpType.add,
            ).then_inc(s_add, 1)

        # ---- PE: per row, 63 col-sum matmuls over the bf16 reduced tile ----
        for n in range(NS):
            nc.tensor.wait_ge(s_add, n + 1)
            for c in range(CH):
                w = min(P, K - c * P)
                mm = nc.tensor.matmul(
                    out=pss[n][:w, c : c + 1],
                    lhsT=accb[n % 2][:, c * P : c * P + w],
                    rhs=ones_bf[:, :1],
                    start=True,
                    stop=True,
                )
                if c == CH - 1:
                    mm.then_inc(pe_sem, 1)

        # ---- ACT: per row, log(colsum/T + eps) from PSUM ----
        for n in range(NS):
            nc.scalar.wait_ge(pe_sem, n + 1)
            nc.scalar.activation(
                out=logt[:, n * CH : n * CH + full],
                in_=pss[n][:, 0:full],
                func=mybir.ActivationFunctionType.Ln,
                bias=SOFT,
                scale=1.0 / T,
            )
            nc.scalar.activation(
                out=logt[:tail, n * CH + full : n * CH + full + 1],
                in_=pss[n][:tail, full : full + 1],
                func=mybir.ActivationFunctionType.Ln,
                bias=SOFT,
                scale=1.0 / T,
            ).then_inc(act_sem, 1)

        # ---- DVE: zero tail partitions, then weighted reduce ----
        for n in range(NS):
            nc.vector.memset(
                logt[tail:P, n * CH + full : n * CH + full + 1], 0.0
            )
        nc.vector.wait_ge(s_w, 16)
        nc.vector.wait_ge(act_sem, NS)
        nc.vector.tensor_tensor(
            out=prod[:], in0=logt[:], in1=wtile[:], op=mybir.AluOpType.mult
        )
        nc.vector.reduce_sum(
            out=acc[:], in_=prod[:], axis=mybir.AxisListType.X
        ).then_inc(s_fin, 1)
    return nc


def _get_nc():
    if "nc" not in _cached:
        _cached["nc"] = _build_nc()
    return _cached["nc"]


def _make_wts(targets_shard: np.ndarray) -> np.ndarray:
    """[NS, L] int -> [P, NS*CH] f32 with w[p, n*CH+c] = -count[n, c*128+p]/(N*T)."""
    w = np.zeros((P, NS * CH), np.float32)
    for n in range(NS):
        cnt = np.bincount(
            targets_shard[n].astype(np.int64), minlength=CH * P
        ).astype(np.float32)
        w[:, n * CH : (n + 1) * CH] = (-cnt / (N * T)).reshape(CH, P).T
    return w


def kernel(**inputs) -> np.ndarray:
    from concourse.bass_utils import run_bass_kernel_spmd

    probs = np.ascontiguousarray(np.asarray(inputs["probs"], dtype=np.float32))
    targets = np.asarray(inputs["targets"])

    nc = _get_nc()
    in_maps = []
    for c in range(NCORES):
        sl = slice(c * NS, (c + 1) * NS)
        in_maps.append(
            {"probs": probs[sl], "wts": _make_wts(np.asarray(targets[sl]))}
        )
    res = run_bass_kernel_spmd(nc, in_maps, core_ids=list(range(NCORES))).results
    total = np.float64(0.0)
    for r in res:
        total += np.sum(np.asarray(r["out"], dtype=np.float64))
    return np.array(total, dtype=np.float32)


# revision 15
# speedup vs baseline: 2.0784x; 1.0507x over previous
"""Trainium2 Bass kernel for histogram-binning NLL loss.

reference:
    probs: [N=32, T=256, K=8000] f32, targets: [N, L=64] int
    agg[n,k]    = sum_t (probs[n,t,k] + 1e-10)       = colsum[n,k] + T*1e-10
    count[n,k]  = histogram(targets[n]) over K
    loss        = mean_n( -sum_k log(agg/T) * count/T )
                = sum_{n,k} (-count[n,k]/(N*T)) * log(colsum[n,k]/T + 1e-10)

Sharding: pure data-parallel over N across 8 cores (4 rows each).
Each core:
  - streams its probs shard [4, 256, 8000] as 8 x [128, 8000] tiles,
    cast f32->bf16 inline in the DMA (SWDGE); all 8 tiles stay resident
  - DVE adds the two T halves per row -> bf16 reduced tile
  - PE: 63 col-sum matmuls per row (bf16 stationary, rhs=ones[128,1])
    -> PSUM colsum laid out K-on-partitions
  - ScalarE: log(colsum/T + 1e-10) from PSUM
  - VectorE: multiply by host-precomputed weights -count/(N*T), reduce
  -> out [128, 1]; host sums the 8 x 128 partials.
"""

import numpy as np

N, T, K = 32, 256, 8000
L = 64
NCORES = 8
NS = N // NCORES  # rows per core = 4
P = 128
CH = (K + P - 1) // P  # 63 column chunks per row
SOFT = 1e-10

_cached = {}


def _build_nc():
    from contextlib import ExitStack

    import concourse.bass as bass
    import concourse.mybir as mybir

    nc = bass.Bass()
    probs = nc.declare_dram_parameter(
        "probs", [NS, T, K], mybir.dt.float32, isOutput=False
    )
    wts = nc.declare_dram_parameter(
        "wts", [P, NS * CH], mybir.dt.float32, isOutput=False
    )
    out = nc.declare_dram_parameter("out", [P, 1], mybir.dt.float32, isOutput=True)

    # Register const APs (behind an all-engine barrier) so matmul rhs /
    # activation bias+scale carry no semaphore deps.
    for v in (SOFT, 1.0 / T):
        t = nc.alloc_sbuf_tensor(f"const-f32-{v}", [128, 1], mybir.dt.float32)
        nc.gpsimd.memset(t.ap(), v)
        nc.const_aps.aps[(mybir.dt.float32, v)] = t.ap()
    nc.all_engine_barrier()
    ones_bf = nc.const_aps.tensor(1.0, (P, 1), mybir.dt.bfloat16)

    # [NS*T, K] -> [NS*2, 128, K] tiles; tile 2n/2n+1 = row n's two T halves
    ptiles = probs[:].rearrange("n (j p) k -> (n j) p k", p=P)

    NT = 2 * NS  # 8 load tiles, all resident (bf16)
    full = CH - 1
    tail = K - full * P

    ctx = ExitStack()
    with ctx:
        bufs = [
            ctx.enter_context(nc.sbuf_tensor(f"buf{i}", [P, K], mybir.dt.bfloat16))
            for i in range(NT)
        ]
        # bf16 per-row T-reduced tiles (DVE add of the two T halves)
        accb = [
            ctx.enter_context(nc.sbuf_tensor(f"accb{i}", [P, K], mybir.dt.bfloat16))
            for i in range(2)
        ]
        wtile = ctx.enter_context(
            nc.sbuf_tensor("wtile", [P, NS * CH], mybir.dt.float32)
        )
        logt = ctx.enter_context(
            nc.sbuf_tensor("logt", [P, NS * CH], mybir.dt.float32)
        )
        prod = ctx.enter_context(
            nc.sbuf_tensor("prod", [P, NS * CH], mybir.dt.float32)
        )
        acc = ctx.enter_context(nc.sbuf_tensor("acc", [P, 1], mybir.dt.float32))
        pss = [
            ctx.enter_context(nc.psum_tensor(f"ps{i}", [P, CH], mybir.dt.float32))
            for i in range(NS)
        ]
        s_buf = [ctx.enter_context(nc.semaphore(f"s_buf{i}")) for i in range(NT)]
        s_w = ctx.enter_context(nc.semaphore("s_w"))
        s_out = ctx.enter_context(nc.semaphore("s_out"))
        s_add = ctx.enter_context(nc.semaphore("s_add"))
        pe_sem = ctx.enter_context(nc.semaphore("pe_sem"))
        act_sem = ctx.enter_context(nc.semaphore("act_sem"))
        s_fin = ctx.enter_context(nc.semaphore("s_fin"))

        # ---- GPSIMD/SWDGE: all probs DMAs with inline f32->bf16 cast ----
        for i in range(NT):
            nc.gpsimd.dma_start(out=bufs[i][:], in_=ptiles[i]).then_inc(
                s_buf[i], 16
            )

        # ---- SYNC: wts load + final out DMA ----
        nc.sync.dma_start(out=wtile[:], in_=wts[:]).then_inc(s_w, 16)
        nc.sync.wait_ge(s_fin, 1)
        nc.sync.dma_start(out=out[:], in_=acc[:]).then_inc(s_out, 16)
        nc.sync.wait_ge(s_out, 16)

        # ---- DVE: per row, add the two T halves (bf16) ----
        for n in range(NS):
            nc.vector.wait_ge(s_buf[2 * n], 16)
            nc.vector.wait_ge(s_buf[2 * n + 1], 16)
            if n >= 2:
                # accb slot reuse: PE must be done with row n-2
                nc.vector.wait_ge(pe_sem, n - 1)
            nc.vector.tensor_tensor(
                out=accb[n % 2][:],
                in0=bufs[2 * n][:],
                in1=bufs[2 * n + 1][:],
                op=mybir.AluOpType.add,
            ).then_inc(s_add, 1)

        # ---- PE: per row, 63 col-sum matmuls over the bf16 reduced tile ----
        for n in range(NS):
            nc.tensor.wait_ge(s_add, n + 1)
            for c in range(CH):
                w = min(P, K - c * P)
                mm = nc.tensor.matmul(
                    out=pss[n][:w, c : c + 1],
                    lhsT=accb[n % 2][:, c * P : c * P + w],
                    rhs=ones_bf[:, :1],
                    start=True,
                    stop=True,
                )
                if c == CH - 1:
                    mm.then_inc(pe_sem, 1)

        # ---- ACT: per row, log(colsum/T + eps) from PSUM ----
        for n in range(NS):
            nc.scalar.wait_ge(pe_sem, n + 1)
            nc.scalar.activation(
                out=logt[:, n * CH : n * CH + full],
                in_=pss[n][:, 0:full],
                func=mybir.ActivationFunctionType.Ln,
                bias=SOFT,
                scale=1.0 / T,
            )
            nc.scalar.activation(
                out=logt[:tail, n * CH + full : n * CH + full + 1],
                in_=pss[n][:tail, full : full + 1],
                func=mybir.ActivationFunctionType.Ln,
                bias=SOFT,
                scale=1.0 / T,
            ).then_inc(act_sem, 1)

        # ---- DVE: zero tail partitions, then weighted reduce ----
        for n in range(NS):
            nc.vector.memset(
                logt[tail:P, n * CH + full : n * CH + full + 1], 0.0
            )
        nc.vector.wait_ge(s_w, 16)
        nc.vector.wait_ge(act_sem, NS)
        nc.vector.tensor_tensor(
            out=prod[:], in0=logt[:], in1=wtile[:], op=mybir.AluOpType.mult
        )
        nc.vector.reduce_sum(
            out=acc[:], in_=prod[:], axis=mybir.AxisListType.X
        ).then_inc(s_fin, 1)
    return nc


def _get_nc():
    if "nc" not in _cached:
        _cached["nc"] = _build_nc()
    return _cached["nc"]


def _make_wts(targets_shard: np.ndarray) -> np.ndarray:
    """[NS, L] int -> [P, NS*CH] f32 with w[p, n*CH+c] = -count[n, c*128+p]/(N*T)."""
    w = np.zeros((P, NS * CH), np.float32)
    for n in range(NS):
        cnt = np.bincount(
            targets_shard[n].astype(np.int64), minlength=CH * P
        ).astype(np.float32)
        w[:, n * CH : (n + 1) * CH] = (-cnt / (N * T)).reshape(CH, P).T
    return w


def kernel(**inputs) -> np.ndarray:
    from concourse.bass_utils import run_bass_kernel_spmd

    probs = np.ascontiguousarray(np.asarray(inputs["probs"], dtype=np.float32))
    targets = np.asarray(inputs["targets"])

    nc = _get_nc()
    in_maps = []
    for c in range(NCORES):
        sl = slice(c * NS, (c + 1) * NS)
        in_maps.append(
            {"probs": probs[sl], "wts": _make_wts(np.asarray(targets[sl]))}
        )
    res = run_bass_kernel_spmd(nc, in_maps, core_ids=list(range(NCORES))).results
    total = np.float64(0.0)
    for r in res:
        total += np.sum(np.asarray(r["out"], dtype=np.float64))
    return np.array(total, dtype=np.float32)


# revision 16
# speedup vs baseline: 2.1712x; 1.0447x over previous
"""Trainium2 Bass kernel for histogram-binning NLL loss.

reference:
    probs: [N=32, T=256, K=8000] f32, targets: [N, L=64] int
    agg[n,k]    = sum_t (probs[n,t,k] + 1e-10)       = colsum[n,k] + T*1e-10
    count[n,k]  = histogram(targets[n]) over K
    loss        = mean_n( -sum_k log(agg/T) * count/T )
                = sum_{n,k} (-count[n,k]/(N*T)) * log(colsum[n,k]/T + 1e-10)

Sharding: pure data-parallel over N across 8 cores (4 rows each).
Each core:
  - streams its probs shard [4, 256, 8000] as 8 x [128, 8000] tiles,
    cast f32->bf16 inline in the DMA (SWDGE); all 8 tiles stay resident
  - DVE adds the two T halves per row -> bf16 reduced tile
  - PE: 63 col-sum matmuls per row (bf16 stationary, rhs=ones[128,1])
    -> PSUM colsum laid out K-on-partitions
  - ScalarE: log(colsum/T + 1e-10) from PSUM
  - VectorE: multiply by host-precomputed weights -count/(N*T), reduce
  -> out [128, 1]; host sums the 8 x 128 partials.
"""

import numpy as np

N, T, K = 32, 256, 8000
L = 64
NCORES = 8
NS = N // NCORES  # rows per core = 4
P = 128
CH = (K + P - 1) // P  # 63 column chunks per row
SOFT = 1e-10

_cached = {}


def _build_nc():
    from contextlib import ExitStack

    import concourse.bass as bass
    import concourse.mybir as mybir

    nc = bass.Bass()
    probs = nc.declare_dram_parameter(
        "probs", [NS, T, K], mybir.dt.float32, isOutput=False
    )
    wts = nc.declare_dram_parameter(
        "wts", [P, NS * CH], mybir.dt.float32, isOutput=False
    )
    out = nc.declare_dram_parameter("out", [P, 1], mybir.dt.float32, isOutput=True)

    # ACT computes plain log(colsum); the /T scale and the +1e-10 soften
    # (which is below f32 ulp at colsum ~ 128) are folded into a host-side
    # constant: loss = sum(w*log(colsum)) + (L/T)*ln(T). Only the
    # pre-registered 0.0/1.0 const APs are needed -> no extra init barrier.
    ones_bf = nc.const_aps.tensor(1.0, (P, 1), mybir.dt.bfloat16)

    # [NS*T, K] -> [NS*2, 128, K] tiles; tile 2n/2n+1 = row n's two T halves
    ptiles = probs[:].rearrange("n (j p) k -> (n j) p k", p=P)

    NT = 2 * NS  # 8 load tiles, all resident (bf16)
    full = CH - 1
    tail = K - full * P

    ctx = ExitStack()
    with ctx:
        bufs = [
            ctx.enter_context(nc.sbuf_tensor(f"buf{i}", [P, K], mybir.dt.bfloat16))
            for i in range(NT)
        ]
        # bf16 per-row T-reduced tiles (DVE add of the two T halves)
        accb = [
            ctx.enter_context(nc.sbuf_tensor(f"accb{i}", [P, K], mybir.dt.bfloat16))
            for i in range(2)
        ]
        wtile = ctx.enter_context(
            nc.sbuf_tensor("wtile", [P, NS * CH], mybir.dt.float32)
        )
        logt = ctx.enter_context(
            nc.sbuf_tensor("logt", [P, NS * CH], mybir.dt.float32)
        )
        prod = ctx.enter_context(
            nc.sbuf_tensor("prod", [P, NS * CH], mybir.dt.float32)
        )
        acc = ctx.enter_context(nc.sbuf_tensor("acc", [P, 1], mybir.dt.float32))
        pss = [
            ctx.enter_context(nc.psum_tensor(f"ps{i}", [P, CH], mybir.dt.float32))
            for i in range(NS)
        ]
        s_buf = [ctx.enter_context(nc.semaphore(f"s_buf{i}")) for i in range(NT)]
        s_w = ctx.enter_context(nc.semaphore("s_w"))
        s_out = ctx.enter_context(nc.semaphore("s_out"))
        s_add = ctx.enter_context(nc.semaphore("s_add"))
        pe_sem = ctx.enter_context(nc.semaphore("pe_sem"))
        act_sem = ctx.enter_context(nc.semaphore("act_sem"))
        s_fin = ctx.enter_context(nc.semaphore("s_fin"))

        # ---- GPSIMD/SWDGE: all probs DMAs with inline f32->bf16 cast ----
        for i in range(NT):
            nc.gpsimd.dma_start(out=bufs[i][:], in_=ptiles[i]).then_inc(
                s_buf[i], 16
            )

        # ---- SYNC: wts load + final out DMA ----
        nc.sync.dma_start(out=wtile[:], in_=wts[:]).then_inc(s_w, 16)
        nc.sync.wait_ge(s_fin, 1)
        nc.sync.dma_start(out=out[:], in_=acc[:]).then_inc(s_out, 16)
        nc.sync.wait_ge(s_out, 16)

        # ---- DVE: per row, add the two T halves (bf16) ----
        for n in range(NS):
            nc.vector.wait_ge(s_buf[2 * n], 16)
            nc.vector.wait_ge(s_buf[2 * n + 1], 16)
            if n >= 2:
                # accb slot reuse: PE must be done with row n-2
                nc.vector.wait_ge(pe_sem, n - 1)
            nc.vector.tensor_tensor(
                out=accb[n % 2][:],
                in0=bufs[2 * n][:],
                in1=bufs[2 * n + 1][:],
                op=mybir.AluOpType.add,
            ).then_inc(s_add, 1)

        # ---- PE: per row, 63 col-sum matmuls over the bf16 reduced tile ----
        for n in range(NS):
            nc.tensor.wait_ge(s_add, n + 1)
            for c in range(CH):
                w = min(P, K - c * P)
                mm = nc.tensor.matmul(
                    out=pss[n][:w, c : c + 1],
                    lhsT=accb[n % 2][:, c * P : c * P + w],
                    rhs=ones_bf[:, :1],
                    start=True,
                    stop=True,
                )
                if c == CH - 1:
                    mm.then_inc(pe_sem, 1)

        # ---- ACT: per row, log(colsum/T + eps) from PSUM ----
        for n in range(NS):
            nc.scalar.wait_ge(pe_sem, n + 1)
            nc.scalar.activation(
                out=logt[:, n * CH : n * CH + full],
                in_=pss[n][:, 0:full],
                func=mybir.ActivationFunctionType.Ln,
                bias=0.0,
                scale=1.0,
            )
            nc.scalar.activation(
                out=logt[:tail, n * CH + full : n * CH + full + 1],
                in_=pss[n][:tail, full : full + 1],
                func=mybir.ActivationFunctionType.Ln,
                bias=0.0,
                scale=1.0,
            ).then_inc(act_sem, 1)

        # ---- DVE: zero tail partitions, then weighted reduce ----
        for n in range(NS):
            nc.vector.memset(
                logt[tail:P, n * CH + full : n * CH + full + 1], 0.0
            )
        nc.vector.wait_ge(s_w, 16)
        nc.vector.wait_ge(act_sem, NS)
        nc.vector.tensor_tensor(
            out=prod[:], in0=logt[:], in1=wtile[:], op=mybir.AluOpType.mult
        )
        nc.vector.reduce_sum(
            out=acc[:], in_=prod[:], axis=mybir.AxisListType.X
        ).then_inc(s_fin, 1)
    return nc


def _get_nc():
    if "nc" not in _cached:
        _cached["nc"] = _build_nc()
    return _cached["nc"]


def _make_wts(targets_shard: np.ndarray) -> np.ndarray:
    """[NS, L] int -> [P, NS*CH] f32 with w[p, n*CH+c] = -count[n, c*128+p]/(N*T)."""
    w = np.zeros((P, NS * CH), np.float32)
    for n in range(NS):
        cnt = np.bincount(
            targets_shard[n].astype(np.int64), minlength=CH * P
        ).astype(np.float32)
        w[:, n * CH : (n + 1) * CH] = (-cnt / (N * T)).reshape(CH, P).T
    return w


def kernel(**inputs) -> np.ndarray:
    from concourse.bass_utils import run_bass_kernel_spmd

    probs = np.ascontiguousarray(np.asarray(inputs["probs"], dtype=np.float32))
    targets = np.asarray(inputs["targets"])

    nc = _get_nc()
    in_maps = []
    for c in range(NCORES):
        sl = slice(c * NS, (c + 1) * NS)
        in_maps.append(
            {"probs": probs[sl], "wts": _make_wts(np.asarray(targets[sl]))}
        )
    res = run_bass_kernel_spmd(nc, in_maps, core_ids=list(range(NCORES))).results
    total = np.float64(0.0)
    for r in res:
        total += np.sum(np.asarray(r["out"], dtype=np.float64))
    # fold back the /T scale dropped on-device: sum(w) * (-ln T) with
    # sum(w) = -L/T  =>  + (L/T) * ln(T)
    total += (L / T) * np.log(np.float64(T))
    return np.array(total, dtype=np.float32)
